# revision 54
# baseline (speedup 1.0000x reference)
"""Fused causal MHA (RoPE) Trainium2 Bass kernel, SPMD over 8 NeuronCores.

Sharding: data-parallel over batch (4) x tensor-parallel over heads (2 groups
of 8 heads).  Core c handles batch c//2, heads (c%2)*8 .. +8.  Each core
computes a partial output (its 8 heads through the row-sharded Wo); the host
sums the two partials per batch.

v2 structure (all fp16 GEMMs, causal-triangle trimmed, software-pipelined):
  1. v = x-stationary proj -> v_aug [s,e] + ones col       (PE + ACT copy)
  2. q/k projection matmuls drip into the attention stream (one per step,
     with per-kt drain checkpoints) so softmax windows keep the PE busy.
     RoPE reads the projection PSUM on DVE (cross-partition sin-muls are
     PSUM-only on HW); the final add runs on the idle Pool engine.
  3. scores per (head, q-half 1024, k-tile 128): matmuls trimmed to the
     causal span and split at PSUM bank boundaries; diagonal tiles get an
     additive -60000 mask via an N=128 identity matmul; exp on ACT.
  4. AV accumulates [65, 1024] (row 64 = softmax denominators); emission
     lags four k-tiles behind the score matmuls to hide exp latency.
  5. normalize: one ACT copy PSUM->SBUF fp16 (releases the PSUM bank),
     then fp16 DVE reciprocal + Pool partition_broadcast + DVE mul.
  6. yT = Wo'^T o  (PE)  -> DVE copy -> DMA fp16; host sums TP partials.
"""

import numpy as np

B = 4
S = 2048
D = 1024
H = 16
DK = 64
THETA = 10000.0
N_CORES = 8
E = 512          # per-core head-dim shard (8 heads * 64)
NHP = 4          # head pairs per core
ST = S // 128    # seq tiles of 128
DC = D // 128    # d_model chunks of 128
QH = 1024        # q processed in halves per head
MASK_NEG = -60000.0

_cache = {}
LABELS = {}


def _lab(inst, label):
    try:
        LABELS[inst.ins.name] = label
    except Exception:
        pass
    return inst


def _build_program():
    import concourse.tile as tile
    from concourse import bacc, mybir
    from contextlib import ExitStack

    f16 = mybir.dt.float16
    f32 = mybir.dt.float32

    nc = bacc.Bacc("TRN2", target_bir_lowering=False, debug=False,
                   num_devices=N_CORES)

    def din(name, shape, dt=f16):
        return nc.dram_tensor(name, shape, dt, kind="ExternalInput").ap()

    xt = din("xt", [128, DC, S])               # x[b]^T as [128, dchunk, s]
    wqt = din("wqt", [128, DC, E])             # Wq' (permuted) ^T
    wkt = din("wkt", [128, DC, E])
    wvt = din("wvt", [128, DC, E])
    wot = din("wot", [128, E // 128, D])       # Wo'^T chunks [128e, 1024d]
    ctab = din("ctab", [128, S])               # cos table (rotate-half layout)
    stab = din("stab", [128, S])               # signed sin table
    madd = din("madd", [128, 128])             # additive causal mask (diag)
    ident = din("ident", [128, 128])
    yt = nc.dram_tensor("yt", [128, DC, S], f16, kind="ExternalOutput").ap()

    with tile.TileContext(nc) as tc, ExitStack() as ctx:
        sb = ctx.enter_context(tc.tile_pool(name="sb", bufs=1))
        rope_tmp = ctx.enter_context(tc.tile_pool(name="rtmp", bufs=6))
        exps_pool = ctx.enter_context(tc.tile_pool(name="exps", bufs=6))
        r_pool = ctx.enter_context(tc.tile_pool(name="rp", bufs=4))
        oc_pool = ctx.enter_context(tc.tile_pool(name="ocp", bufs=4))
        rb_pool = ctx.enter_context(tc.tile_pool(name="rbp", bufs=4))

        # ---- resident SBUF tensors ----
        s_xt = sb.tile([128, DC, S], f16)
        s_wq = sb.tile([128, DC, E], f16)
        s_wk = sb.tile([128, DC, E], f16)
        s_wv = sb.tile([128, DC, E], f16)
        s_wo = sb.tile([128, E // 128, D], f16)
        s_c = sb.tile([128, S], f16)
        s_s = sb.tile([128, S], f16)
        s_m = sb.tile([128, 128], f16)
        s_id = sb.tile([128, 128], f16)
        v_aug = sb.tile([128, ST, 8, 72], f16)   # [k-part, ktile, head, dv+ones]
        q_all = sb.tile([128, NHP, S], f16)
        k_all = sb.tile([128, NHP, S], f16)
        o_all = sb.tile([128, NHP, S], f16)      # normalized attn out (e-major)

        # per-chunk DMAs so compute can start as soon as early chunks land;
        # order: wv + first halves of xt (phase B start), then the rest
        for d in range(DC):
            nc.sync.dma_start(s_wv[:, d, :], wvt[:, d, :])
            nc.sync.dma_start(s_xt[:, d, 0:1024], xt[:, d, 0:1024])
        for d in range(DC):
            nc.sync.dma_start(s_xt[:, d, 1024:2048], xt[:, d, 1024:2048])
        for d in range(DC):
            nc.sync.dma_start(s_wq[:, d, :], wqt[:, d, :])
            nc.sync.dma_start(s_wk[:, d, :], wkt[:, d, :])
        for dst, src in [(s_c, ctab), (s_s, stab), (s_m, madd),
                         (s_id, ident), (s_wo, wot)]:
            nc.sync.dma_start(dst[:], src[:])

        # ones column for softmax denominators
        nc.vector.memset(
            v_aug.rearrange("p a h c -> p (a h) c")[:, :, 64:65], 1.0)

        # ---- q/k projection units, consumed lazily by the attention loop --
        def rope(ps_p, out_slice, sc_i):
            # 4-block rows per head pair: [A_x1, A_x2, B_x1, B_x2] (32 each);
            # the swapped-row sin-muls must read PSUM (SB-SB cross-partition
            # reads are illegal on HW).
            cs = s_c[:, sc_i * 512:(sc_i + 1) * 512]
            ss = s_s[:, sc_i * 512:(sc_i + 1) * 512]
            t = rope_tmp.tile([128, 512], f16, tag="ropet", name="ropet")
            m = rope_tmp.tile([128, 512], f16, tag="ropem", name="ropem")
            nc.vector.tensor_mul(t[:], ps_p[:], cs)
            for g in range(4):
                src = 32 * (g ^ 1)
                nc.vector.tensor_mul(m[32 * g:32 * (g + 1), :],
                                     ps_p[src:src + 32, :],
                                     ss[32 * g:32 * (g + 1), :])
            nc.gpsimd.tensor_add(out_slice, t[:], m[:])

        with tc.tile_pool(name="pqk", bufs=2, space="PSUM") as pqk, \
             tc.tile_pool(name="psc", bufs=2, space="PSUM") as psc, \
             tc.tile_pool(name="po", bufs=1, space="PSUM") as po:

            units_done = [0]

            def proj_steps():
                """Yield after each PE matmul; bump units_done per unit.

                Unit order matches attention consumption: (hp, sc, q|k).
                """
                for hp in range(NHP):
                    for qh_o in range(2):
                        order = [(2 * qh_o, s_wq, q_all),
                                 (2 * qh_o + 1, s_wq, q_all),
                                 (2 * qh_o, s_wk, k_all),
                                 (2 * qh_o + 1, s_wk, k_all)]
                        for sc_i, w, dst in order:
                            ps_p = pqk.tile([128, 512], f32, name="ps_p")
                            for d in range(DC):
                                _lab(nc.tensor.matmul(
                                    ps_p[:],
                                    w[:, d, hp * 128:(hp + 1) * 128],
                                    s_xt[:, d, sc_i * 512:(sc_i + 1) * 512],
                                    start=(d == 0), stop=(d == DC - 1)),
                                    "proj-mm")
                                yield
                            rope(ps_p,
                                 dst[:, hp, sc_i * 512:(sc_i + 1) * 512],
                                 sc_i)
                            units_done[0] += 1
                while True:
                    yield

            proj = proj_steps()
            mm_pulled = [0]

            def pull_mm():
                next(proj)
                mm_pulled[0] += 1

            def drain_until(n_units):
                while units_done[0] < n_units:
                    next(proj)

            # ---- phase B: v projection (x stationary -> [s, e] layout),
            # sharing the proj PSUM ring; the prime projection units
            # interleave into the DMA-paced tail of the v phase ----
            for st_i in range(ST):
                ps_v = pqk.tile([128, E], f32, tag="ps_p", name="ps_p")
                for d in range(DC):
                    _lab(nc.tensor.matmul(
                        ps_v[:], s_xt[:, d, st_i * 128:(st_i + 1) * 128],
                        s_wv[:, d, :], start=(d == 0), stop=(d == DC - 1)),
                        "v-mm")
                nc.scalar.activation(
                    out=v_aug[:, st_i, :, 0:64],
                    in_=ps_v.rearrange("p (h v) -> p h v", h=8),
                    func=mybir.ActivationFunctionType.Copy)
                if st_i >= 12:
                    for _ in range(DC):
                        pull_mm()

            # prime: projections for (hp0, sc0..1, q+k) = 4 units
            drain_until(4)
            mm_pulled[0] = 32
            kt_step = [0]
            TOT_STEPS = 192.0

            # ---- attention: per (head pair, q-half, head) with
            #      score-lookahead and interleaved projection matmuls ----
            for hp in range(NHP):
                for qh in range(2):
                    base = hp * 8 + 4 * qh
                    for h in range(2):
                        nkt = 8 * (qh + 1)
                        ps_o = po.tile([128, QH], f32, name="ps_o")
                        qs = q_all[64 * h:64 * (h + 1), hp,
                                   qh * QH:(qh + 1) * QH]
                        # group k-tiles into psc tiles: wide spans (>512) get
                        # their own tile; tail spans pack two per tile at the
                        # 512-column bank boundary, sharing one exp.
                        groups, cur = [], []
                        for kt in range(nkt):
                            cs = max(0, 128 * kt - QH * qh)
                            span = QH - cs
                            if span > 512:
                                groups.append([(kt, cs, 0, span)])
                            elif not cur:
                                cur = [(kt, cs, 0, span)]
                            else:
                                cur.append((kt, cs, 512, span))
                                groups.append(cur)
                                cur = []
                        if cur:
                            groups.append(cur)

                        pend = []     # (kt, cs, off, span, ex) awaiting AV
                        def emit_av(ent, last):
                            pkt, pcs, poff, pspan, pex = ent
                            for ob in range(0, QH, 512):
                                lo = max(pcs, ob)
                                hi = min(pcs + pspan, ob + 512)
                                if lo >= hi:
                                    continue
                                _lab(nc.tensor.matmul(
                                    ps_o[0:65, lo:hi],
                                    v_aug[:, pkt, 2 * hp + h, 0:65],
                                    pex[:, poff + lo - pcs:poff + hi - pcs],
                                    start=(pkt == 0), stop=last,
                                    skip_group_check=True), "av-mm")

                        for grp in groups:
                            ps_s = psc.tile([128, QH], f32, tag="ps_s",
                                            name="ps_s")
                            ex = exps_pool.tile([128, QH], f16, tag="ex",
                                                name="ex")
                            for kt, cs, off, span in grp:
                                if qh == 0:
                                    drain_until(base + (3 if kt < 4 else 4))
                                else:
                                    drain_until(base + (2 if kt < 8 else
                                                        3 if kt < 12 else 4))
                                pull_mm()
                                if qh == 0:
                                    pull_mm()
                                diag = kt >= 8 * qh
                                kv = k_all[64 * h:64 * (h + 1), hp,
                                           kt * 128:(kt + 1) * 128]
                                for b0 in range(off, off + span, 512):
                                    c1 = min(b0 + 512, off + span)
                                    _lab(nc.tensor.matmul(
                                        ps_s[:, b0:c1], kv,
                                        qs[:, cs + b0 - off:cs + c1 - off],
                                        start=True,
                                        stop=not (diag and b0 == off),
                                        skip_group_check=True), "score-mm")
                                if diag:
                                    _lab(nc.tensor.matmul(
                                        ps_s[:, off:off + 128],
                                        s_id[:], s_m[:],
                                        start=False, stop=True,
                                        skip_group_check=True), "mask-mm")
                            g_end = grp[-1][2] + grp[-1][3]
                            nc.scalar.activation(
                                out=ex[:, 0:g_end], in_=ps_s[:, 0:g_end],
                                func=mybir.ActivationFunctionType.Exp,
                                scale=0.125)
                            for kt, cs, off, span in grp:
                                pend.append((kt, cs, off, span, ex))
                            while len(pend) > 4:
                                emit_av(pend.pop(0), last=False)
                        for i, ent in enumerate(pend):
                            emit_av(ent, last=(i == len(pend) - 1))
                        # single ACT copy releases the po bank; the rest of
                        # the normalize runs on fp16 SBUF (2x DVE mode)
                        oc = oc_pool.tile([65, QH], f16, tag="oc", name="oc")
                        nc.scalar.activation(
                            out=oc[:], in_=ps_o[0:65, :],
                            func=mybir.ActivationFunctionType.Copy)
                        r = r_pool.tile([1, QH], f16, tag="recip", name="r")
                        with nc.allow_low_precision(
                                reason="fp16 softmax denom recip, 5e-4 rel"):
                            nc.vector.reciprocal(r[:], oc[64:65, :])
                        rb = rb_pool.tile([64, QH], f16, tag="rbc", name="rb")
                        nc.gpsimd.partition_broadcast(rb[:], r[:])
                        nc.vector.tensor_mul(
                            o_all[64 * h:64 * (h + 1), hp,
                                  qh * QH:(qh + 1) * QH],
                            oc[0:64, :], rb[:])


        # ---- phase E: output projection ----
        with tc.tile_pool(name="pf", bufs=2, space="PSUM") as pf, \
             tc.tile_pool(name="yst", bufs=3) as yst:
            for mt in range(DC):
                for sh in range(2):
                    ps = pf.tile([128, QH], f32)
                    for ec in range(E // 128):
                        for c0 in range(0, QH, 512):
                            _lab(nc.tensor.matmul(
                                ps[:, c0:c0 + 512],
                                s_wo[:, ec, mt * 128:(mt + 1) * 128],
                                o_all[:, ec,
                                      sh * QH + c0:sh * QH + c0 + 512],
                                start=(ec == 0), stop=(ec == E // 128 - 1),
                                skip_group_check=True), "wo-mm")
                    yo = yst.tile([128, QH], f16)
                    nc.vector.tensor_copy(yo[:], ps[:])
                    nc.sync.dma_start(
                        yt[:, mt, sh * QH:(sh + 1) * QH], yo[:])

    nc.compile()
    return nc


def _prepare_inputs(x, wq, wk, wv, wo, token_positions):
    """Build the 8 per-core input maps (all host-side layout shuffling)."""
    x = np.asarray(x, dtype=np.float32)
    wq = np.asarray(wq, dtype=np.float32)
    wk = np.asarray(wk, dtype=np.float32)
    wv = np.asarray(wv, dtype=np.float32)
    wo = np.asarray(wo, dtype=np.float32)
    pos = np.asarray(token_positions).astype(np.float32)

    # RoPE tables in rotate-half row layout (32 freqs tiled 4x over 128 rows)
    inv = THETA ** (-np.arange(0, DK, 2, dtype=np.float32) / DK)  # [32]
    ang = pos[:, None] * inv[None, :]                             # [S, 32]
    cosT = np.cos(ang).T.astype(np.float32)                       # [32, S]
    sinT = np.sin(ang).T.astype(np.float32)
    ctab = np.tile(cosT, (4, 1)).astype(np.float16)               # [128, S]
    stab = np.concatenate([-sinT, sinT, -sinT, sinT], 0).astype(np.float16)

    # additive causal mask for the diagonal 128x128 blocks: kill q < k
    kk = np.arange(128)[:, None]
    qq = np.arange(128)[None, :]
    madd = np.where(qq < kk, MASK_NEG, 0.0).astype(np.float16)    # [128, 128]
    identity = np.eye(128, dtype=np.float16)

    def chunk_T(a, inner):
        # [rows, cols] -> transpose -> [128, rows/... ] device chunk layout
        t = a.T  # [cols, rows]
        n = t.shape[0] // 128
        return np.ascontiguousarray(
            t.reshape(n, 128, t.shape[1]).transpose(1, 0, 2))

    def perm_rows(w, hf):
        # per head pair: [A_evens, A_odds, B_evens, B_odds] (32 rows each)
        out = np.empty((E, D), dtype=np.float32)
        for hp in range(NHP):
            hA = hf * 8 + 2 * hp
            hB = hA + 1
            out[128 * hp + 0:128 * hp + 32] = w[64 * hA + 0:64 * (hA + 1):2]
            out[128 * hp + 32:128 * hp + 64] = w[64 * hA + 1:64 * (hA + 1):2]
            out[128 * hp + 64:128 * hp + 96] = w[64 * hB + 0:64 * (hB + 1):2]
            out[128 * hp + 96:128 * hp + 128] = w[64 * hB + 1:64 * (hB + 1):2]
        return out

    in_maps = []
    for core in range(N_CORES):
        b, hf = divmod(core, 2)
        xtc = chunk_T(x[b], DC).astype(np.float16)         # [128, 8, 2048]
        wqtc = chunk_T(perm_rows(wq, hf), DC).astype(np.float16)
        wktc = chunk_T(perm_rows(wk, hf), DC).astype(np.float16)
        wvtc = chunk_T(wv[E * hf:E * (hf + 1)], DC).astype(np.float16)
        # Wo'^T: rows e in shard, cols d -> chunks [128, 4, 1024]
        woT = wo.T[E * hf:E * (hf + 1)]                    # [512, 1024]
        wotc = np.ascontiguousarray(
            woT.reshape(4, 128, D).transpose(1, 0, 2)).astype(np.float16)
        in_maps.append({
            "xt": xtc, "wqt": wqtc, "wkt": wktc, "wvt": wvtc, "wot": wotc,
            "ctab": ctab, "stab": stab, "madd": madd, "ident": identity,
        })
    return in_maps


def _assemble(results):
    out = np.zeros((B, S, D), dtype=np.float32)
    for core, res in enumerate(results):
        b = core // 2
        part = res["yt"].astype(np.float32)
        part = part.transpose(1, 0, 2).reshape(D, S)       # [1024, 2048]
        out[b] += part.T
    return out


def get_program():
    if "nc" not in _cache:
        _cache["nc"] = _build_program()
    return _cache["nc"]


def kernel(x, wq, wk, wv, wo, token_positions):
    from concourse.bass_utils import run_bass_kernel_spmd

    nc = get_program()
    in_maps = _prepare_inputs(x, wq, wk, wv, wo, token_positions)
    res = run_bass_kernel_spmd(nc, in_maps, core_ids=list(range(N_CORES)))
    return _assemble(res.results)
